# revision 11
# baseline (speedup 1.0000x reference)
"""Trainium2 Bass kernel for nn_DecoderGQALayer (GQA attention + top-2 MoE).

Sharding (8 NeuronCores):
  - Attention: group-parallel. Core c computes Q/K/V + causal attention for
    GQA group g=c (both batches), producing the d-slice [c*128:(c+1)*128] of
    x1 = x + attn_out, in transposed layout x1T [d, tok].
  - x1T slices are AllGathered on-device (1 MB/rank).
  - MoE: expert-parallel. Core c runs expert e=c densely over all 2048
    tokens, weighted by its gate column. Host sums the 8 partial outputs
    and adds x1 (core 0 ships x1T back).
  - Norms / router / gates / aux computed redundantly on every core.

All activations live in transposed [d, token] layout so every matmul's
contraction dim sits on SBUF partitions with zero on-device transposes
(except the softmax-prob tiles, which use PE transposes).

Expert matmuls run in float32r (TF32-like, full PE rate at N>=512);
everything feeding the router runs in plain fp32 so top-2 selection is
bit-faithful to the fp32 reference.
"""

import sys

if "/opt/trn_rl_repo" not in sys.path:
    sys.path.insert(0, "/opt/trn_rl_repo")

import numpy as np

import concourse.bass as bass
import concourse.tile as tile
from concourse import bacc, mybir
from concourse.bass import ds, ts
from concourse.bass_utils import run_bass_kernel_spmd

F32 = mybir.dt.float32
F32R = mybir.dt.float32r
AF = mybir.ActivationFunctionType
ALU = mybir.AluOpType
AX = mybir.AxisListType

B, S, D = 2, 1024, 1024
G, H, HD = 8, 2, 64
E, TOPK, FF = 8, 2, 4 * D
T = B * S                      # 2048 tokens
N_CORES = 8
ROPE_THETA = 10000.0
EPS = 1e-6
NEG_BIG = -1.0e30

DO = D // 128                  # 8   d-outer tiles
TT = T // 128                  # 16  token tiles
FO = FF // 128                 # 32  ff-outer tiles
N_FFB = 8                      # ff blocks for the MoE sweep
FPB = FO // N_FFB              # 4   ff-outer tiles per block
STT = S // 128                 # 8   s-tiles per batch


def _r32(ap):
    return ap.bitcast(F32R)


def build_bass():
    nc = bacc.Bacc("TRN2", target_bir_lowering=False, debug=False,
                   num_devices=N_CORES)

    # ---- I/O ----
    xT = nc.dram_tensor("xT", [D, T], F32, kind="ExternalInput")
    x_sliceT = nc.dram_tensor("x_sliceT", [128, T], F32, kind="ExternalInput")
    wq = nc.dram_tensor("wq", [D, H * HD], F32, kind="ExternalInput")
    wk2 = nc.dram_tensor("wk2", [D, 2 * HD], F32, kind="ExternalInput")
    wv = nc.dram_tensor("wv", [D, HD], F32, kind="ExternalInput")
    w2T = nc.dram_tensor("w2T", [128, DO], F32, kind="ExternalInput")
    router_w = nc.dram_tensor("router_w", [D, E], F32, kind="ExternalInput")
    wg = nc.dram_tensor("wg", [D, FF], F32R, kind="ExternalInput")
    wu = nc.dram_tensor("wu", [D, FF], F32R, kind="ExternalInput")
    wd = nc.dram_tensor("wd", [FF, D], F32R, kind="ExternalInput")
    ccT = nc.dram_tensor("ccT", [128, T], F32, kind="ExternalInput")
    ssT = nc.dram_tensor("ssT", [128, T], F32, kind="ExternalInput")
    tri = nc.dram_tensor("tri", [128, 128], F32, kind="ExternalInput")
    ident = nc.dram_tensor("ident", [128, 128], F32, kind="ExternalInput")
    ones2d = nc.dram_tensor("ones2d", [128, 128], F32, kind="ExternalInput")
    eoh = nc.dram_tensor("eoh", [128, E], F32, kind="ExternalInput")

    y_out = nc.dram_tensor("y_out", [T, D], F32, kind="ExternalOutput")
    x1T_out = nc.dram_tensor("x1T_out", [D, T], F32, kind="ExternalOutput")
    aux_out = nc.dram_tensor("aux_out", [1, 1], F32, kind="ExternalOutput")

    xT_r = xT.rearrange("(do p) t -> p do t", p=128)
    rw_r = router_w.rearrange("(ko p) e -> p ko e", p=128)
    wg_r = wg.rearrange("(ko p) f -> p ko f", p=128)
    wu_r = wu.rearrange("(ko p) f -> p ko f", p=128)
    wd_r = wd.rearrange("(ko p) d -> p ko d", p=128)
    y_r = y_out.rearrange("(tt p) d -> p tt d", p=128)

    with tile.TileContext(nc) as tc:
        consts = tc.alloc_tile_pool(name="consts", bufs=1)
        dram = tc.alloc_tile_pool(name="dram", bufs=1, space="DRAM")

        ones_sb = consts.tile([128, 128], F32)
        nc.sync.dma_start(ones_sb[:], ones2d[:])
        ident_sb = consts.tile([128, 128], F32)
        nc.sync.dma_start(ident_sb[:], ident[:])
        tri_sb = consts.tile([128, 128], F32)
        nc.sync.dma_start(tri_sb[:], tri[:])
        eoh_sb = consts.tile([128, E], F32)
        nc.sync.dma_start(eoh_sb[:], eoh[:])
        w2T_sb = consts.tile([128, DO], F32)
        nc.sync.dma_start(w2T_sb[:], w2T[:])
        xsl_sb = consts.tile([128, T], F32)
        nc.sync.dma_start(xsl_sb[:], x_sliceT[:])
        eps_sb = consts.tile([1, 1], F32)
        nc.vector.memset(eps_sb[:], EPS)

        # =============================================================
        # helper: rms scale from a [128, DO, T] transposed tile
        #   returns (s_row [1,T], s_bcast [128,T]) in `pool`
        # =============================================================
        def rms_scale(src_sb, scratch_pool, pspool, tag):
            s_row = scratch_pool.tile([1, T], F32, tag=f"srow_{tag}")
            for ch in range(T // 512):
                sq = scratch_pool.tile([128, DO, 512], F32, tag=f"sq_{tag}")
                nc.vector.tensor_mul(sq[:], src_sb[:, :, ds(ch * 512, 512)],
                                     src_sb[:, :, ds(ch * 512, 512)])
                ps = pspool.tile([1, 512], F32, tag="ssq_ps")
                for ko in range(DO):
                    nc.tensor.matmul(
                        ps[:], ones_sb[:, 0:1], sq[:, ko, :],
                        start=(ko == 0), stop=(ko == DO - 1),
                    )
                # sqrt(ssq/D + eps)
                nc.scalar.activation(
                    s_row[:, ds(ch * 512, 512)], ps[:], AF.Sqrt,
                    bias=eps_sb[:], scale=1.0 / D,
                )
            nc.vector.reciprocal(s_row[:], s_row[:])
            s_bc = scratch_pool.tile([128, T], F32, tag=f"sbc_{tag}")
            nc.gpsimd.partition_broadcast(s_bc[:], s_row[:])
            return s_row, s_bc

        # =============================================================
        # Stage 1+2: load xT, norm1 stats, QKV projections + RoPE
        # =============================================================
        persist = tc.alloc_tile_pool(name="persist", bufs=1)

        attn = tc.alloc_tile_pool(name="attn", bufs=1)
        s12 = tc.alloc_tile_pool(name="stage12", bufs=1)
        pA = tc.alloc_tile_pool(name="psA", bufs=3, space="PSUM")

        xT_sb = s12.tile([128, DO, T], F32)
        nc.sync.dma_start(xT_sb[:], xT_r[:])

        s_row, s_bc = rms_scale(xT_sb, s12, pA, "n1")

        # s_col [128, TT]: s per token in column form (via DRAM bounce)
        s_bounce = dram.tile([1, T], F32)
        nc.sync.dma_start(s_bounce[:], s_row[:])
        s_col = s12.tile([128, TT], F32)
        nc.sync.dma_start(
            s_col[:], s_bounce.rearrange("o (j p) -> p (o j)", p=128))

        # scaled rope tables (stacked to 128 rows): fold rms scale in
        cc_sb = attn.tile([128, T], F32, tag="cc")
        ss_sb = attn.tile([128, T], F32, tag="ss")
        nc.sync.dma_start(cc_sb[:], ccT[:])
        nc.sync.dma_start(ss_sb[:], ssT[:])
        nc.vector.tensor_mul(cc_sb[:], cc_sb[:], s_bc[:])
        nc.vector.tensor_mul(ss_sb[:], ss_sb[:], s_bc[:])

        wq_sb = s12.tile([128, DO, H * HD], F32)
        nc.sync.dma_start(wq_sb[:], wq.rearrange("(ko p) m -> p ko m", p=128))
        wk_sb = s12.tile([128, DO, 2 * HD], F32)
        nc.sync.dma_start(wk_sb[:], wk2.rearrange("(ko p) m -> p ko m", p=128))
        wv_sb = s12.tile([128, DO, HD], F32)
        nc.sync.dma_start(wv_sb[:], wv.rearrange("(ko p) m -> p ko m", p=128))

        qT_raw = attn.tile([128, T], F32, tag="qT_raw")
        kT_raw = attn.tile([128, T], F32, tag="kT_raw")
        qT = attn.tile([128, T], F32, tag="qT")
        kT = attn.tile([128, T], F32, tag="kT")
        v_sb = attn.tile([128, TT, HD], F32, tag="v")

        for ch in range(T // 512):
            qps = pA.tile([128, 512], F32, tag="qkv_ps")
            for ko in range(DO):
                nc.tensor.matmul(
                    qps[:], wq_sb[:, ko, :], xT_sb[:, ko, ds(ch * 512, 512)],
                    start=(ko == 0), stop=(ko == DO - 1))
            nc.vector.tensor_copy(qT_raw[:, ds(ch * 512, 512)], qps[:])
            kps = pA.tile([128, 512], F32, tag="qkv_ps")
            for ko in range(DO):
                nc.tensor.matmul(
                    kps[:], wk_sb[:, ko, :], xT_sb[:, ko, ds(ch * 512, 512)],
                    start=(ko == 0), stop=(ko == DO - 1))
            nc.vector.tensor_copy(kT_raw[:, ds(ch * 512, 512)], kps[:])

        for tt in range(TT):
            vps_full = pA.tile([128, 512], F32, tag="qkv_ps")
            vps = vps_full[:, :HD]
            for ko in range(DO):
                nc.tensor.matmul(
                    vps[:], xT_sb[:, ko, ts(tt, 128)], wv_sb[:, ko, :],
                    start=(ko == 0), stop=(ko == DO - 1))
            # fold rms scale (per-token = per-partition here) into v
            nc.scalar.activation(v_sb[:, tt, :], vps[:], AF.Copy,
                                 scale=s_col[:, tt:tt + 1])

        # RoPE via half-swap (SBUF->SBUF DMA crosses partitions) so every
        # DVE op is full-width and partition-aligned:
        #   rot = raw * cc + swap(raw) * ss
        # with cc = [cos]*4 stacked, ss = [-sin, +sin]*2 stacked.
        def rope(dst, src_t, swp):
            for r0 in (0, 64):
                nc.sync.dma_start(swp[r0:r0 + 32, :], src_t[r0 + 32:r0 + 64, :])
                nc.sync.dma_start(swp[r0 + 32:r0 + 64, :], src_t[r0:r0 + 32, :])
            nc.vector.tensor_mul(dst[:], src_t[:], cc_sb[:])
            nc.vector.tensor_mul(swp[:], swp[:], ss_sb[:])
            nc.vector.tensor_add(dst[:], dst[:], swp[:])

        qsw = attn.tile([128, T], F32, tag="qsw")
        rope(qT, qT_raw, qsw)
        # kT_raw rows 64:127 already duplicate rows 0:63 (wk2 is tiled 2x)
        ksw = attn.tile([128, T], F32, tag="ksw")
        rope(kT, kT_raw, ksw)

        pA.release()
        s12.release()   # frees xT + weight tiles

        # =============================================================
        # Stage 3: causal attention per (b, h); x1T slice
        # =============================================================
        x1c_sb = persist.tile([128, T], F32, tag="x1c")

        psS = tc.alloc_tile_pool(name="psS", bufs=2, space="PSUM")
        psTr = tc.alloc_tile_pool(name="psTr", bufs=2, space="PSUM")
        psO = tc.alloc_tile_pool(name="psO", bufs=1, space="PSUM")
        ap2 = tc.alloc_tile_pool(name="apool", bufs=2)
        pTpool = tc.alloc_tile_pool(name="pTpool", bufs=1)

        for b in range(B):
            oT_ps = psO.tile([128, S], F32, tag="oT")
            for h in range(H):
                r0 = h * 64
                pT_sb = pTpool.tile([128, STT, S], F32, tag="pT")
                for si in range(STT):
                    L = (si + 1) * 128
                    sc = ap2.tile([128, S], F32, tag="scores")
                    for c0 in range(0, L, 512):
                        w = min(512, L - c0)
                        sps = psS.tile([128, 512], F32, tag="s_ps")
                        nc.tensor.matmul(
                            sps[:, :w],
                            qT[r0:r0 + 64, ds(b * S + si * 128, 128)],
                            kT[r0:r0 + 64, ds(b * S + c0, w)],
                            start=True, stop=True)
                        nc.scalar.activation(sc[:, ds(c0, w)], sps[:, :w],
                                             AF.Copy, scale=1.0 / 8.0)
                    nc.vector.tensor_add(sc[:, ts(si, 128)],
                                         sc[:, ts(si, 128)], tri_sb[:])
                    mx = ap2.tile([128, 1], F32, tag="mx")
                    nc.vector.reduce_max(mx[:], sc[:, :L], axis=AX.X)
                    nc.vector.tensor_scalar_mul(mx[:], mx[:], -1.0)
                    ssum = ap2.tile([128, 1], F32, tag="ssum")
                    p_t = ap2.tile([128, S], F32, tag="p")
                    nc.scalar.activation(p_t[:, :L], sc[:, :L], AF.Exp,
                                         bias=mx[:], accum_out=ssum[:])
                    nc.vector.reciprocal(ssum[:], ssum[:])
                    nc.vector.tensor_scalar_mul(p_t[:, :L], p_t[:, :L],
                                                ssum[:])
                    for tj in range(si + 1):
                        tps = psTr.tile([128, 128], F32, tag="tr_ps")
                        nc.tensor.transpose(tps[:], p_t[:, ts(tj, 128)],
                                            ident_sb[:])
                        nc.vector.tensor_copy(pT_sb[:, tj, ts(si, 128)],
                                              tps[:])
                # AV: oT[r0:r0+64, s] = sum_tj v[tj].T @ pT[tj]
                for k2 in range(S // 512):
                    lo = k2 * 512
                    for tj in range(min(4 * k2 + 4, STT)):
                        sub = max(0, tj * 128 - lo)
                        w = 512 - sub
                        last = (tj == min(4 * k2 + 3, STT - 1))
                        nc.tensor.matmul(
                            oT_ps[r0:r0 + 64, ds(lo + sub, w)],
                            v_sb[:, b * STT + tj, :],
                            pT_sb[:, tj, ds(lo + sub, w)],
                            start=(tj == 0), stop=last,
                            tile_position=(0, r0))
            # x1T slice = xT slice + attn_outT  (both heads done)
            nc.vector.tensor_add(x1c_sb[:, ds(b * S, S)],
                                 xsl_sb[:, ds(b * S, S)], oT_ps[:])

        for p_ in (pTpool, ap2, psO, psTr, psS):
            p_.release()

        # =============================================================
        # Stage 4: AllGather x1T slices
        # =============================================================
        ag_in = dram.tile([128, T], F32)
        ag_out = dram.tile([D, T], F32)
        nc.sync.dma_start(ag_in[:], x1c_sb[:])
        attn.release()
        nc.gpsimd.collective_compute(
            "AllGather", ALU.bypass,
            replica_groups=[list(range(N_CORES))],
            ins=[ag_in.opt()], outs=[ag_out.opt()])
        nc.sync.dma_start(x1T_out[:], ag_out[:])

        # =============================================================
        # Stage 5: norm2, router, gates, aux
        # =============================================================
        h2pool = tc.alloc_tile_pool(name="h2", bufs=1)
        n2 = tc.alloc_tile_pool(name="n2", bufs=1)

        p5 = tc.alloc_tile_pool(name="ps5", bufs=2, space="PSUM")
        x1T_sb = n2.tile([128, DO, T], F32)
        nc.sync.dma_start(x1T_sb[:], ag_out.rearrange("(do p) t -> p do t",
                                                      p=128))
        s2_row, s2_bc = rms_scale(x1T_sb, n2, p5, "n2")

        h2r = h2pool.tile([128, DO, T], F32R)
        nc.vector.tensor_mul(
            h2r[:], x1T_sb[:],
            s2_bc[:, None, :].to_broadcast((128, DO, T)))
        nc.vector.tensor_mul(
            h2r[:], h2r[:],
            w2T_sb[:, :, None].to_broadcast((128, DO, T)))

        # s2 in column form for the router logit scaling
        s2_bounce = dram.tile([1, T], F32)
        nc.sync.dma_start(s2_bounce[:], s2_row[:])
        s2_col = n2.tile([128, TT], F32)
        nc.sync.dma_start(
            s2_col[:], s2_bounce.rearrange("o (j p) -> p (o j)", p=128))

        rpool = tc.alloc_tile_pool(name="router", bufs=1)
        rw_sb = rpool.tile([128, DO, E], F32)
        nc.sync.dma_start(rw_sb[:], rw_r[:])
        logits = rpool.tile([128, TT, E], F32)
        for tt in range(TT):
            rps = p5.tile([128, E], F32, tag="r_ps")
            for ko in range(DO):
                nc.tensor.matmul(rps[:], x1T_sb[:, ko, ts(tt, 128)],
                                 rw_sb[:, ko, :],
                                 start=(ko == 0), stop=(ko == DO - 1))
            nc.scalar.activation(logits[:, tt, :], rps[:], AF.Copy,
                                 scale=s2_col[:, tt:tt + 1])

        # softmax over E (free dim)
        mx2 = rpool.tile([128, TT], F32, tag="mx2")
        nc.vector.reduce_max(mx2[:], logits[:], axis=AX.X)
        probs = rpool.tile([128, TT, E], F32, tag="probs")
        nc.vector.tensor_sub(probs[:], logits[:],
                             mx2[:, :, None].to_broadcast((128, TT, E)))
        nc.scalar.activation(probs[:], probs[:], AF.Exp)
        se = rpool.tile([128, TT], F32, tag="se")
        nc.vector.reduce_sum(se[:], probs[:], axis=AX.X)
        nc.vector.reciprocal(se[:], se[:])
        nc.vector.tensor_mul(probs[:], probs[:],
                             se[:, :, None].to_broadcast((128, TT, E)))

        # top-2 gates
        m1 = rpool.tile([128, TT], F32, tag="m1")
        nc.vector.reduce_max(m1[:], probs[:], axis=AX.X)
        tmp = rpool.tile([128, TT, E], F32, tag="tmp8")
        nc.vector.tensor_tensor(tmp[:], probs[:],
                                m1[:, :, None].to_broadcast((128, TT, E)),
                                ALU.is_ge)
        nc.vector.tensor_mul(tmp[:], tmp[:], probs[:])
        p2 = rpool.tile([128, TT, E], F32, tag="p2")
        nc.vector.tensor_sub(p2[:], probs[:], tmp[:])
        m2 = rpool.tile([128, TT], F32, tag="m2")
        nc.vector.reduce_max(m2[:], p2[:], axis=AX.X)
        sel = rpool.tile([128, TT, E], F32, tag="sel")
        nc.vector.tensor_tensor(sel[:], probs[:],
                                m2[:, :, None].to_broadcast((128, TT, E)),
                                ALU.is_ge)
        den = rpool.tile([128, TT], F32, tag="den")
        nc.vector.tensor_add(den[:], m1[:], m2[:])
        nc.vector.reciprocal(den[:], den[:])
        gates_all = rpool.tile([128, TT, E], F32, tag="gall")
        nc.vector.tensor_mul(gates_all[:], probs[:], sel[:])
        nc.vector.tensor_mul(gates_all[:], gates_all[:],
                             den[:, :, None].to_broadcast((128, TT, E)))
        gates_c = persist.tile([128, TT], F32, tag="gates_c")
        gsel = rpool.tile([128, TT, E], F32, tag="gsel")
        nc.vector.tensor_mul(gsel[:], gates_all[:],
                             eoh_sb[:, None, :].to_broadcast((128, TT, E)))
        nc.vector.reduce_sum(gates_c[:], gsel[:], axis=AX.X)

        # aux loss: E/( (B*S*K)*(B*S) ) * sum_e selsum_e * probssum_e
        selsum = rpool.tile([128, E], F32, tag="selsum")
        nc.vector.reduce_sum(selsum[:], sel.rearrange("p t e -> p e t"),
                             axis=AX.X)
        psum_p = rpool.tile([128, E], F32, tag="psum_p")
        nc.vector.reduce_sum(psum_p[:], probs.rearrange("p t e -> p e t"),
                             axis=AX.X)
        fps = p5.tile([1, E], F32, tag="aux_ps")
        nc.tensor.matmul(fps[:], ones_sb[:, 0:1], selsum[:],
                         start=True, stop=True)
        pps = p5.tile([1, E], F32, tag="aux_ps")
        nc.tensor.matmul(pps[:], ones_sb[:, 0:1], psum_p[:],
                         start=True, stop=True)
        fsb = rpool.tile([1, E], F32, tag="fsb")
        nc.vector.tensor_copy(fsb[:], fps[:])
        nc.vector.tensor_tensor(fsb[:], fsb[:], pps[:], ALU.mult)
        auxv = rpool.tile([1, 1], F32, tag="auxv")
        nc.vector.reduce_sum(auxv[:], fsb[:], axis=AX.X)
        aux_sb = rpool.tile([1, 1], F32, tag="aux_sb")
        nc.scalar.activation(aux_sb[:], auxv[:], AF.Copy,
                             scale=float(E) / (T * TOPK * T))
        nc.sync.dma_start(aux_out[:], aux_sb[:])

        rpool.release()
        p5.release()
        n2.release()   # frees x1T + sq

        # =============================================================
        # Stage 6: expert MoE (f32r matmuls), ff-blocked, DRAM-accum
        # =============================================================
        y_acc = dram.tile([T, D], F32)
        y_acc_r = y_acc.rearrange("(tt p) d -> p tt d", p=128)

        moew = tc.alloc_tile_pool(name="moew", bufs=3)
        moewd = tc.alloc_tile_pool(name="moewd", bufs=2)
        moea = tc.alloc_tile_pool(name="moea", bufs=1)
        yts = tc.alloc_tile_pool(name="yts", bufs=3)
        psG = tc.alloc_tile_pool(name="psG", bufs=2, space="PSUM")
        psU = tc.alloc_tile_pool(name="psU", bufs=2, space="PSUM")
        psD = tc.alloc_tile_pool(name="psD", bufs=2, space="PSUM")

        for blk in range(N_FFB):
            f0 = blk * FPB * 128       # ff offset of block
            actT = moea.tile([128, FPB, T], F32R, tag="actT")
            for fo in range(FPB):
                wg_sb = moew.tile([128, DO, 128], F32R, tag="wg")
                nc.sync.dma_start(wg_sb[:], wg_r[:, :, ds(f0 + fo * 128, 128)])
                wu_sb = moew.tile([128, DO, 128], F32R, tag="wu")
                nc.sync.dma_start(wu_sb[:], wu_r[:, :, ds(f0 + fo * 128, 128)])
                for ch in range(T // 512):
                    gps = psG.tile([128, 512], F32, tag="g_ps")
                    for ko in range(DO):
                        nc.tensor.matmul(
                            gps[:], wg_sb[:, ko, :],
                            h2r[:, ko, ds(ch * 512, 512)],
                            start=(ko == 0), stop=(ko == DO - 1))
                    nc.scalar.activation(actT[:, fo, ds(ch * 512, 512)],
                                         gps[:], AF.Silu)
                    ups = psU.tile([128, 512], F32, tag="u_ps")
                    for ko in range(DO):
                        nc.tensor.matmul(
                            ups[:], wu_sb[:, ko, :],
                            h2r[:, ko, ds(ch * 512, 512)],
                            start=(ko == 0), stop=(ko == DO - 1))
                    nc.vector.tensor_mul(actT[:, fo, ds(ch * 512, 512)],
                                         actT[:, fo, ds(ch * 512, 512)],
                                         ups[:])
            wd_sb = moewd.tile([128, FPB, D], F32R, tag="wd")
            nc.sync.dma_start(wd_sb[:], wd_r[:, ds(blk * FPB, FPB), :])
            wd_tiles = [wd_sb[:, kb, :] for kb in range(FPB)]
            for tt in range(TT):
                for n_ in range(D // 512):
                    dps = psD.tile([128, 512], F32, tag="d_ps")
                    for kb in range(FPB):
                        nc.tensor.matmul(
                            dps[:], actT[:, kb, ts(tt, 128)],
                            wd_tiles[kb][:, ds(n_ * 512, 512)],
                            start=(kb == 0), stop=(kb == FPB - 1))
                    yt = yts.tile([128, 512], F32, tag="yt")
                    nc.scalar.activation(yt[:], dps[:], AF.Copy,
                                         scale=gates_c[:, tt:tt + 1])
                    nc.gpsimd.dma_start(
                        y_acc_r[:, tt, ds(n_ * 512, 512)], yt[:],
                        accum_op=(ALU.bypass if blk == 0 else ALU.add))

        nc.sync.dma_start(y_out[:], y_acc[:])

        for p_ in (psD, psU, psG, yts, moea, moewd, moew, h2pool):
            p_.release()
        for p_ in (persist, dram, consts):
            p_.release()

    nc.finalize()
    return nc


_CACHED = None


def _get_bass():
    global _CACHED
    if _CACHED is None:
        _CACHED = build_bass()
    return _CACHED


def make_in_maps(inputs):
    x = np.asarray(inputs["x"], np.float32)
    norm1_w = np.asarray(inputs["norm1_w"], np.float32)
    Wq = np.asarray(inputs["Wq"], np.float32)
    Wk = np.asarray(inputs["Wk"], np.float32)
    Wv = np.asarray(inputs["Wv"], np.float32)
    norm2_w = np.asarray(inputs["norm2_w"], np.float32)
    router_w = np.asarray(inputs["router_w"], np.float32)
    w_gate = np.asarray(inputs["w_gate"], np.float32)
    w_up = np.asarray(inputs["w_up"], np.float32)
    w_down = np.asarray(inputs["w_down"], np.float32)

    xT = np.ascontiguousarray(x.reshape(T, D).T)          # [D, T]

    inv = 1.0 / (ROPE_THETA ** (np.arange(0, HD, 2, dtype=np.float32) / HD))
    ang = np.arange(S, dtype=np.float32)[:, None] * inv[None, :]   # [S, 32]
    cos32 = np.concatenate([np.cos(ang).T, np.cos(ang).T], axis=1)  # [32, T]
    sin32 = np.concatenate([np.sin(ang).T, np.sin(ang).T], axis=1)
    ccT = np.ascontiguousarray(np.tile(cos32, (4, 1))).astype(np.float32)
    ssT = np.ascontiguousarray(
        np.tile(np.concatenate([-sin32, sin32], axis=0), (2, 1))).astype(np.float32)

    tri = np.where(np.arange(128)[:, None] >= np.arange(128)[None, :],
                   0.0, NEG_BIG).astype(np.float32)
    ident = np.eye(128, dtype=np.float32)
    ones2d = np.ones((128, 128), np.float32)
    w2T = np.ascontiguousarray(norm2_w.reshape(DO, 128).T)

    in_maps = []
    for c in range(N_CORES):
        eoh = np.zeros((128, E), np.float32)
        eoh[:, c] = 1.0
        in_maps.append({
            "xT": xT,
            "x_sliceT": np.ascontiguousarray(xT[c * 128:(c + 1) * 128]),
            "wq": np.ascontiguousarray(Wq[c] * norm1_w[:, None]),
            "wk2": np.ascontiguousarray(
                np.tile(Wk[c] * norm1_w[:, None], (1, 2))),
            "wv": np.ascontiguousarray(Wv[c] * norm1_w[:, None]),
            "w2T": w2T,
            "router_w": np.ascontiguousarray(
                router_w * norm2_w[:, None]),
            "wg": np.ascontiguousarray(w_gate[c]),
            "wu": np.ascontiguousarray(w_up[c]),
            "wd": np.ascontiguousarray(w_down[c]),
            "ccT": ccT,
            "ssT": ssT,
            "tri": tri,
            "ident": ident,
            "ones2d": ones2d,
            "eoh": eoh,
        })
    return in_maps


def run(inputs, trace=False, **kw):
    nc = _get_bass()
    in_maps = make_in_maps(inputs)
    res = run_bass_kernel_spmd(nc, in_maps, core_ids=list(range(N_CORES)),
                               trace=trace, **kw)
    y = res.results[0]["x1T_out"].astype(np.float64).T.copy()
    for c in range(N_CORES):
        y += res.results[c]["y_out"].astype(np.float64)
    y = y.astype(np.float32).reshape(B, S, D)
    aux = np.float32(res.results[0]["aux_out"][0, 0])
    return (y, np.asarray(aux)), res


def kernel(**inputs):
    out, _ = run(inputs, trace=False)
    return out
